# revision 1
# baseline (speedup 1.0000x reference)
"""Multi-head causal attention with RoPE on 8 Trainium2 NeuronCores.

Sharding: core = batch(2) x head-group(4).  Each core computes the q/k/v
projections for its 4 heads (256 of 1024 channels), RoPE, causal attention,
and a partial o_proj against its 256 rows of Wo^T; the host sums the 4
partials per batch element.

Device layouts (per core):
  xT       [1024, 2048] f32r   x[b].T
  wqT/wkT/wvT [128, 8*256] f32r  K-block-major W.T slices (wq pre-scaled 1/8)
  woT      [128, 2*1024] bf16  c-block-major Wo[:, g].T
  cosT2/sinT2 [128, 2048] f32r  rope tables, stacked twice (head pair rows)
  rotT     [128, 128]  f32r    blockdiag(R,R).T, R = rotate_half matrix
  triu/ident [128, 128] bf16
  out      [2048, 1024] f32    partial (x @ Wo_g partial), host-summed

Attention per head h (Dh=64): scoresT tiles [s_k 128, s_q 1024] = kT.T@qT
(fp32r), exp -> bf16 sbuf, attn@v natural via ones-column in v (softmax
denominator rides along as column 64 of the psum), per-partition normalize,
PE-transpose of attn_out, o_proj in bf16.
"""
import os
import sys

sys.path.insert(0, "/opt/trn_rl_repo")

import numpy as np
import ml_dtypes

import concourse.bacc as bacc
import concourse.mybir as mybir
from concourse import tile
from concourse.bass_utils import run_bass_kernel_spmd

F32 = mybir.dt.float32
F32R = mybir.dt.float32r
BF16 = mybir.dt.bfloat16

D_MODEL = 1024
N_HEADS = 16
HEAD_DIM = 64
SEQ = 2048
BATCH = 2
ROPE_THETA = 10000.0

NB = SEQ // 128          # 16 s-blocks of 128
NSUP = SEQ // 1024       # 2 s-supers of 1024
HPG = 4                  # heads per group (per core)
CPG = HPG * HEAD_DIM     # 256 channels per group

_CACHE = {}
LAST_RESULT = None       # test harness reads exec_time_ns from here


def _build_nc(causal: bool):
    nc = bacc.Bacc("TRN2", target_bir_lowering=False, debug=False, num_devices=8)

    xT_d = nc.declare_dram_parameter("xT", [D_MODEL, SEQ], F32R, isOutput=False)
    wq_d = nc.declare_dram_parameter("wqT", [128, 8 * CPG], F32R, isOutput=False)
    wk_d = nc.declare_dram_parameter("wkT", [128, 8 * CPG], F32R, isOutput=False)
    wv_d = nc.declare_dram_parameter("wvT", [128, 8 * CPG], F32R, isOutput=False)
    wo_d = nc.declare_dram_parameter("woT", [128, 2 * D_MODEL], BF16, isOutput=False)
    cos_d = nc.declare_dram_parameter("cosT2", [128, SEQ], F32R, isOutput=False)
    sin_d = nc.declare_dram_parameter("sinT2", [128, SEQ], F32R, isOutput=False)
    rot_d = nc.declare_dram_parameter("rotT", [128, 128], F32R, isOutput=False)
    tri_d = nc.declare_dram_parameter("triu", [128, 128], BF16, isOutput=False)
    id_d = nc.declare_dram_parameter("ident", [128, 128], BF16, isOutput=False)
    wn_d = nc.declare_dram_parameter("wneg", [128, 1024], BF16, isOutput=False)
    out_d = nc.declare_dram_parameter("out", [D_MODEL, SEQ], F32, isOutput=True)

    xT_r = xT_d.rearrange("(kb p) s -> p kb s", p=128)

    with tile.TileContext(nc) as tc:
        with (
            tc.tile_pool(name="res", bufs=1) as res,
            tc.tile_pool(name="ps", bufs=8, space="PSUM") as ps,
        ):
            # ---- resident constants ----
            wq_sb = res.tile([128, 8 * CPG], F32R)
            wk_sb = res.tile([128, 8 * CPG], F32R)
            wv_sb = res.tile([128, 8 * CPG], F32R)
            wo_sb = res.tile([128, 2 * D_MODEL], BF16)
            cos_sb = res.tile([128, SEQ], F32R)
            sin_sb = res.tile([128, SEQ], F32R)
            rot_sb = res.tile([128, 128], F32R)
            tri_sb = res.tile([128, 128], BF16)
            id_sb = res.tile([128, 128], BF16)
            nc.sync.dma_start(wq_sb[:], wq_d[:])

            # ---- resident activations ----
            qf = res.tile([128, 2 * SEQ], F32R)          # [pair rows, pr*SEQ + s]
            kf = res.tile([128, 2 * SEQ], F32R)
            v_sb = res.tile([128, NB, HPG * 65], BF16)   # per s-block, head-slot 65 cols
            attn = res.tile([128, NB, CPG], BF16)        # attn out, natural [s, c]
            attnT = res.tile([128, 2 * SEQ], BF16)       # attn out transposed [c, cb*SEQ + s]
            nc.vector.memset(v_sb[:, :, 64 : HPG * 65 : 65], 1.0)

            # prewarm the ACT exp table during the DMA/proj phase
            warm = res.tile([128, 1], F32)
            warm2 = res.tile([128, 1], BF16)
            nc.vector.memset(warm[:], 0.0)
            nc.scalar.activation(warm2[:], warm[:], mybir.ActivationFunctionType.Exp)

            # ================= projections + rope =================
            # all psum tiles are single-bank [*, <=512] f32 in one 8-slot tag
            with tc.tile_pool(name="proj", bufs=2) as proj:
                for sup in range(NSUP):
                    s0 = sup * 1024
                    xp = []
                    for kb in range(8):
                        xt = proj.tile([128, 1024], F32R, name=f"xt{sup}_{kb}", tag="xt", bufs=17)
                        nc.sync.dma_start(xt[:], xT_r[:, kb, s0 : s0 + 1024])
                        xp.append(xt)
                    if sup == 0:
                        # stream the remaining constants behind the first xT tiles
                        # (ordered by first use) so the first projection matmul
                        # starts ~wq+one-tile into the kernel instead of ~6.5MB in
                        nc.sync.dma_start(wk_sb[:], wk_d[:])
                        nc.sync.dma_start(rot_sb[:], rot_d[:])
                        nc.sync.dma_start(cos_sb[:], cos_d[:])
                        nc.sync.dma_start(sin_sb[:], sin_d[:])
                        nc.sync.dma_start(wv_sb[:], wv_d[:])
                        nc.sync.dma_start(tri_sb[:], tri_d[:])
                        nc.sync.dma_start(id_sb[:], id_d[:])
                        nc.sync.dma_start(wo_sb[:], wo_d[:])
                    for tens, (w_sb, outf) in enumerate(((wq_sb, qf), (wk_sb, kf))):
                        # emit both pairs' projection chains before either pair's
                        # rotation, so the rot matmul never blocks the in-order PE
                        # queue waiting on the DVE psum->sbuf copy
                        qraws = []
                        for pr in range(2):
                            qraw = proj.tile([128, 1024], F32R, name="qraw", tag="qraw", bufs=3)
                            for nh in range(2):
                                psq = ps.tile([128, 512], F32, name="psq", tag="pb")
                                for kb in range(8):
                                    lhs = w_sb[:, kb * CPG + pr * 128 : kb * CPG + (pr + 1) * 128]
                                    nc.tensor.matmul(
                                        psq[:],
                                        lhs,
                                        xp[kb][:, nh * 512 : (nh + 1) * 512],
                                        start=(kb == 0),
                                        stop=(kb == 7),
                                    )
                                nc.vector.tensor_copy(qraw[:, nh * 512 : (nh + 1) * 512], psq[:])
                            qraws.append(qraw)
                        for pr in range(2):
                            qraw = qraws[pr]
                            for nh in range(2):
                                psr = ps.tile([128, 512], F32, name="psr", tag="pb")
                                nc.tensor.matmul(
                                    psr[:],
                                    rot_sb[:],
                                    qraw[:, nh * 512 : (nh + 1) * 512],
                                    start=True,
                                    stop=True,
                                )
                                c0 = s0 + nh * 512
                                t1 = proj.tile([128, 512], F32R, name="t1", tag="t1", bufs=3)
                                nc.vector.tensor_mul(
                                    t1[:], qraw[:, nh * 512 : (nh + 1) * 512], cos_sb[:, c0 : c0 + 512]
                                )
                                t2 = proj.tile([128, 512], F32R, name="t2", tag="t2", bufs=3)
                                nc.vector.tensor_mul(t2[:], psr[:], sin_sb[:, c0 : c0 + 512])
                                dst = outf[:, pr * SEQ + c0 : pr * SEQ + c0 + 512]
                                nc.vector.tensor_add(dst, t1[:], t2[:])
                    for sbi in range(8):
                        blk = sup * 8 + sbi
                        psv = ps.tile([128, CPG], F32, name="psv", tag="pb")
                        for kb in range(8):
                            nc.tensor.matmul(
                                psv[:],
                                xp[kb][:, sbi * 128 : (sbi + 1) * 128],
                                wv_sb[:, kb * CPG : (kb + 1) * CPG],
                                start=(kb == 0),
                                stop=(kb == 7),
                            )
                        nc.vector.tensor_copy(
                            v_sb[:, blk, :].rearrange("p (h c) -> p h c", h=HPG)[:, :, 0:64],
                            psv[:].rearrange("p (h c) -> p h c", h=HPG),
                        )

            # ================= attention =================
            # scoresT [s_k 128, s_q 512-half] fp32r + PE diag mask -> exp -> bf16 et
            # -> attn@v transposed per half (ones-column denominators in row 64)
            # -> PE transpose back -> normalize.  The first few score tiles of the
            # NEXT group are emitted before the current group's attn@v block so the
            # ACT exp pipeline stays fed while the PE runs the (independent) block.
            with tc.tile_pool(name="att", bufs=1) as att:
                groups = [(h, J) for h in range(HPG) for J in range(2)]
                prev_tail = None

                def emit_score_tile(h, J, i):
                    pr, off = h // 2, (h % 2) * 64
                    qT_h = qf[off : off + 64, pr * SEQ : (pr + 1) * SEQ]
                    kT_h = kf[off : off + 64, pr * SEQ : (pr + 1) * SEQ]
                    t = i - 8 * J
                    col0 = max(t, 0) * 128 if causal else 0
                    et = att.tile([128, 1024], BF16, name=f"et{h}_{J}_{i}", tag="et", bufs=36)
                    for nh in range(2):
                        lo = max(col0, nh * 512)
                        hi = (nh + 1) * 512
                        if lo >= hi:
                            continue
                        has_mask = causal and t >= 0 and nh == col0 // 512
                        pss = ps.tile([128, 512], F32, name="pss", tag="pb")
                        nc.tensor.matmul(
                            pss[:],
                            kT_h[:, i * 128 : (i + 1) * 128],
                            qT_h[:, J * 1024 + nh * 512 : J * 1024 + (nh + 1) * 512],
                            start=True,
                            stop=not has_mask,
                        )
                        if has_mask:
                            m0 = col0 - nh * 512
                            nc.tensor.matmul(
                                pss[:, m0 : m0 + 128],
                                id_sb[:],
                                tri_sb[:],
                                start=False,
                                stop=True,
                                skip_group_check=True,
                            )
                        nc.scalar.activation(
                            et[:, lo:hi],
                            pss[:, lo - nh * 512 : 512],
                            mybir.ActivationFunctionType.Exp,
                        )
                    if causal and 0 < t <= 3:
                        nc.gpsimd.memset(et[:, 0:col0], 0.0)
                    elif causal and t >= 5:
                        nc.gpsimd.memset(et[:, 512:col0], 0.0)
                    return et

                def make_tail(h, J, n_i, exps):
                    def tail():
                        uoT = att.tile([65, 1024], BF16, name=f"uoT{h}{J}", tag="uoT", bufs=3)
                        n_nh = [
                            min(n_i, 8 * J + 4 * (nh + 1)) if causal else n_i for nh in range(2)
                        ]
                        psuos = [
                            ps.tile([65, 512], F32, name=f"psuo{nh}", tag="pb") for nh in range(2)
                        ]
                        for i in range(max(n_nh)):
                            for nh in range(2):
                                if i < n_nh[nh]:
                                    nc.tensor.matmul(
                                        psuos[nh][:],
                                        v_sb[:, i, h * 65 : h * 65 + 65],
                                        exps[i][:, nh * 512 : (nh + 1) * 512],
                                        start=(i == 0),
                                        stop=(i == n_nh[nh] - 1),
                                    )
                        for nh in range(2):
                            nc.vector.tensor_copy(uoT[:, nh * 512 : (nh + 1) * 512], psuos[nh][:])
                        for t in range(8):
                            j = 8 * J + t
                            pnat = ps.tile([128, 65], BF16, name="pnat", tag="pb")
                            nc.tensor.transpose(
                                pnat[:], uoT[:, t * 128 : (t + 1) * 128], id_sb[0:65, 0:65]
                            )
                            rec = att.tile([128, 1], F32, name="rec", tag="rec", bufs=4)
                            nc.vector.reciprocal(rec[:], pnat[:, 64:65])
                            nc.vector.tensor_scalar_mul(
                                attn[:, j, h * 64 : (h + 1) * 64], pnat[:, 0:64], rec[:]
                            )

                    return tail

                for h, J in groups:
                    n_i = 8 * J + 8 if causal else NB
                    K = min(6, n_i)
                    exps = [emit_score_tile(h, J, i) for i in range(K)]
                    if prev_tail is not None:
                        prev_tail()
                    exps += [emit_score_tile(h, J, i) for i in range(K, n_i)]
                    prev_tail = make_tail(h, J, n_i, exps)
                prev_tail()

            # ================= o_proj =================
            # transpose attn to [c, s] resident, then weight-stationary matmuls:
            # out_pT[d 128, s 512] += woT[c, d-block].T @ attnT[c, s-super],
            # accumulated over the 2 c-blocks; output is [d, s], host transposes.
            with tc.tile_pool(name="oo", bufs=2) as oo:
                for j in range(NB):
                    for cb in range(2):
                        ptt = ps.tile([128, 128], BF16, name="ptt", tag="pb")
                        nc.tensor.transpose(ptt[:], attn[:, j, cb * 128 : (cb + 1) * 128], id_sb[:])
                        nc.vector.tensor_copy(
                            attnT[:, cb * SEQ + j * 128 : cb * SEQ + (j + 1) * 128], ptt[:]
                        )
                for db in range(8):
                    psos = [
                        ps.tile([128, 512], F32, name=f"pso{db}_{ss}", tag="pb")
                        for ss in range(4)
                    ]
                    for cb in range(2):
                        lhs = wo_sb[:, cb * D_MODEL + db * 128 : cb * D_MODEL + (db + 1) * 128]
                        for ss in range(4):
                            nc.tensor.matmul(
                                psos[ss][:],
                                lhs,
                                attnT[:, cb * SEQ + ss * 512 : cb * SEQ + (ss + 1) * 512],
                                start=(cb == 0),
                                stop=(cb == 1),
                            )
                    osb = oo.tile([128, 2048], F32, name="osb", tag="osb", bufs=2)
                    for ss in range(4):
                        if ss % 2 == 0:
                            nc.vector.tensor_copy(osb[:, ss * 512 : (ss + 1) * 512], psos[ss][:])
                        else:
                            nc.scalar.copy(osb[:, ss * 512 : (ss + 1) * 512], psos[ss][:])
                    nc.sync.dma_start(out_d[db * 128 : (db + 1) * 128, :], osb[:])

    nc.compile()
    return nc


def _host_tables():
    inv_freq = 1.0 / (ROPE_THETA ** (np.arange(0, HEAD_DIM, 2, dtype=np.float64) / HEAD_DIM))
    ang = np.arange(SEQ, dtype=np.float64)[:, None] * inv_freq[None, :]  # [S, 32]
    cos_h = np.cos(ang)
    sin_h = np.sin(ang)
    cos_full = np.concatenate([cos_h, cos_h], axis=1).astype(np.float32)  # [S, 64]
    sin_full = np.concatenate([sin_h, sin_h], axis=1).astype(np.float32)
    cosT2 = np.ascontiguousarray(np.vstack([cos_full.T, cos_full.T]))  # [128, S]
    sinT2 = np.ascontiguousarray(np.vstack([sin_full.T, sin_full.T]))
    # rotate_half matrix R [64,64]: (Rq)[j] = -q[j+32] (j<32), q[j-32] (j>=32)
    R = np.zeros((64, 64), np.float32)
    for jj in range(32):
        R[jj, jj + 32] = -1.0
        R[jj + 32, jj] = 1.0
    Rp = np.zeros((128, 128), np.float32)
    Rp[0:64, 0:64] = R
    Rp[64:128, 64:128] = R
    rotT = np.ascontiguousarray(Rp.T)
    return cosT2, sinT2, rotT


def _kb_major(wT):
    # [1024, C] -> [128, 8*C] with K-block-major columns
    C = wT.shape[1]
    return np.ascontiguousarray(wT.reshape(8, 128, C).transpose(1, 0, 2).reshape(128, 8 * C))


def _np_reference(x, mask, Wq, Wk, Wv, Wo):
    B, S, D = x.shape
    cosT2, sinT2, _ = _host_tables()
    cos = cosT2[:64].T[None, :, None, :]  # [1,S,1,64]
    sin = sinT2[:64].T[None, :, None, :]
    q = (x @ Wq.T).reshape(B, S, N_HEADS, HEAD_DIM)
    k = (x @ Wk.T).reshape(B, S, N_HEADS, HEAD_DIM)
    v = (x @ Wv.T).reshape(B, S, N_HEADS, HEAD_DIM)

    def rot(t):
        return np.concatenate([-t[..., 32:], t[..., :32]], axis=-1)

    q = q * cos + rot(q) * sin
    k = k * cos + rot(k) * sin
    sc = np.einsum("bqhd,bkhd->bhqk", q, k) / np.sqrt(HEAD_DIM)
    sc = np.where(mask[None, None], -np.inf, sc)
    sc = sc - sc.max(-1, keepdims=True)
    e = np.exp(sc)
    a = e / e.sum(-1, keepdims=True)
    o = np.einsum("bhqk,bkhd->bqhd", a, v).reshape(B, S, D)
    return (o @ Wo.T).astype(np.float32)


def kernel(x, mask, Wq, Wk, Wv, Wo):
    global LAST_RESULT
    x = np.asarray(x, np.float32)
    mask = np.asarray(mask, bool)
    Wq = np.asarray(Wq, np.float32)
    Wk = np.asarray(Wk, np.float32)
    Wv = np.asarray(Wv, np.float32)
    Wo = np.asarray(Wo, np.float32)

    causal_mask = np.triu(np.ones((SEQ, SEQ), bool), 1)
    if np.array_equal(mask, causal_mask):
        causal = True
    elif not mask.any():
        causal = False
    else:
        return _np_reference(x, mask, Wq, Wk, Wv, Wo)

    if causal not in _CACHE:
        _CACHE[causal] = _build_nc(causal)
    nc = _CACHE[causal]

    cosT2, sinT2, rotT = _host_tables()
    # additive mask for the diagonal 128-block: 0 where q>=k (col>=row), -1e4 else
    triu = np.where(
        np.arange(128)[None, :] >= np.arange(128)[:, None], 0.0, -1.0e4
    ).astype(ml_dtypes.bfloat16)
    ident = np.eye(128, dtype=np.float32).astype(ml_dtypes.bfloat16)
    # wneg: cols 0..895 all -1e4; cols 896..1023 = additive diag mask
    wneg = np.full((128, 1024), -1.0e4, np.float32)
    wneg[:, 896:1024] = np.where(
        np.arange(128)[None, :] >= np.arange(128)[:, None], 0.0, -1.0e4
    )
    wneg = wneg.astype(ml_dtypes.bfloat16)

    in_maps = []
    for b in range(BATCH):
        xT = np.ascontiguousarray(x[b].T)
        for g in range(4):
            sl = slice(g * CPG, (g + 1) * CPG)
            in_maps.append(
                {
                    "xT": xT,
                    "wqT": _kb_major(np.ascontiguousarray((Wq[sl] / np.sqrt(HEAD_DIM)).T)),
                    "wkT": _kb_major(np.ascontiguousarray(Wk[sl].T)),
                    "wvT": _kb_major(np.ascontiguousarray(Wv[sl].T)),
                    "woT": np.ascontiguousarray(
                        Wo[:, sl].T.reshape(2, 128, D_MODEL).transpose(1, 0, 2).reshape(128, 2 * D_MODEL)
                    ).astype(ml_dtypes.bfloat16),
                    "cosT2": cosT2,
                    "sinT2": sinT2,
                    "rotT": rotT,
                    "triu": triu,
                    "wneg": wneg,
                    "ident": ident,
                }
            )

    trace = os.environ.get("KERNEL_TRACE", "0") == "1"
    res = run_bass_kernel_spmd(nc, in_maps, list(range(8)), trace=trace)
    LAST_RESULT = res

    out = np.zeros((BATCH, SEQ, D_MODEL), np.float32)
    for b in range(BATCH):
        for g in range(4):
            out[b] += res.results[b * 4 + g]["out"].T
    return out



# revision 2
# speedup vs baseline: 1.1113x; 1.1113x over previous
"""Multi-head causal attention with RoPE on 8 Trainium2 NeuronCores.

Sharding: core = batch(2) x head-group(4).  Each core computes the q/k/v
projections for its 4 heads (256 of 1024 channels), RoPE, causal attention,
and a partial o_proj against its 256 rows of Wo^T; the host sums the 4
partials per batch element.

Everything is bf16 on the wire and in SBUF (psum accumulation stays f32):
halves HBM traffic vs f32r and enables PE fast-weight-load.

Device layouts (per core):
  xT       [1024, 2048] bf16   x[b].T
  wqT/wkT/wvT [128, 8*256] bf16  K-block-major W.T slices (wq pre-scaled 1/8)
  woT      [128, 2*1024] bf16  c-block-major Wo[:, g].T
  cosT2/sinT2 [128, 2048] bf16 rope tables, stacked twice (head pair rows)
  rotT     [128, 128]  bf16    blockdiag(R,R).T, R = rotate_half matrix
  tri01/ident [128, 128] bf16  lower-incl-diag 0/1 mask; identity
  out      [2048, 1024] f32    partial (x @ Wo_g partial), host-summed

Attention per head h (Dh=64): scoresT tiles [s_k 128, s_q <=1024] = kT.T@qT,
causal-trimmed at 128 cols, exp -> bf16 et sbuf, diag block masked by a 0/1
triangle multiply on gpsimd.  attn@v runs in NATURAL orientation: per q-block
jg, out[q 128, 65] accumulates matmul(lhsT=et[i][:, q-block], rhs=v[:, i, h])
over k-blocks i<=jg -- exact causal trim, full 128-partition output, ones
column of v carries the softmax denominator.  Normalize straight from psum.
o_proj runs per 1024-seq super, interleaved with the other super's attention.
"""
import os
import sys

sys.path.insert(0, "/opt/trn_rl_repo")

import numpy as np
import ml_dtypes

import concourse.bacc as bacc
import concourse.mybir as mybir
from concourse import tile
from concourse.bass_utils import run_bass_kernel_spmd

F32 = mybir.dt.float32
BF16 = mybir.dt.bfloat16

D_MODEL = 1024
N_HEADS = 16
HEAD_DIM = 64
SEQ = 2048
BATCH = 2
ROPE_THETA = 10000.0

NB = SEQ // 128          # 16 s-blocks of 128
NSUP = SEQ // 1024       # 2 s-supers of 1024
HPG = 4                  # heads per group (per core)
CPG = HPG * HEAD_DIM     # 256 channels per group

_CACHE = {}
LAST_RESULT = None       # test harness reads exec_time_ns from here


def _build_nc(causal: bool):
    nc = bacc.Bacc("TRN2", target_bir_lowering=False, debug=False, num_devices=8)

    xT_d = nc.declare_dram_parameter("xT", [D_MODEL, SEQ], BF16, isOutput=False)
    wq_d = nc.declare_dram_parameter("wqT", [128, 8 * CPG], BF16, isOutput=False)
    wk_d = nc.declare_dram_parameter("wkT", [128, 8 * CPG], BF16, isOutput=False)
    wv_d = nc.declare_dram_parameter("wvT", [128, 8 * CPG], BF16, isOutput=False)
    wo_d = nc.declare_dram_parameter("woT", [128, 2 * D_MODEL], BF16, isOutput=False)
    cos_d = nc.declare_dram_parameter("cosT2", [128, SEQ], BF16, isOutput=False)
    sin_d = nc.declare_dram_parameter("sinT2", [128, SEQ], BF16, isOutput=False)
    rot_d = nc.declare_dram_parameter("rotT", [128, 128], BF16, isOutput=False)
    tri_d = nc.declare_dram_parameter("tri01", [128, 128], BF16, isOutput=False)
    id_d = nc.declare_dram_parameter("ident", [128, 128], BF16, isOutput=False)
    out_d = nc.declare_dram_parameter("out", [D_MODEL, SEQ], F32, isOutput=True)

    xT_r = xT_d.rearrange("(kb p) s -> p kb s", p=128)

    with tile.TileContext(nc) as tc:
        with (
            tc.tile_pool(name="res", bufs=1) as res,
            tc.tile_pool(name="ps", bufs=5, space="PSUM") as ps,
        ):
            # ---- resident constants ----
            wq_sb = res.tile([128, 8 * CPG], BF16)
            wk_sb = res.tile([128, 8 * CPG], BF16)
            wv_sb = res.tile([128, 8 * CPG], BF16)
            wo_sb = res.tile([128, 2 * D_MODEL], BF16)
            cos_sb = res.tile([128, SEQ], BF16)
            sin_sb = res.tile([128, SEQ], BF16)
            rot_sb = res.tile([128, 128], BF16)
            tri_sb = res.tile([128, 128], BF16)
            id_sb = res.tile([128, 128], BF16)

            # ---- resident activations ----
            qf = res.tile([128, 2 * SEQ], BF16)          # [pair rows, pr*SEQ + s]
            kf = res.tile([128, 2 * SEQ], BF16)
            v_sb = res.tile([128, NB, HPG * 65], BF16)   # per s-block, head-slot 65 cols
            attn = res.tile([128, NB, CPG], BF16)        # attn out, natural [s, c]
            attnT = res.tile([128, 2 * SEQ], BF16)       # attn out transposed [c, cb*SEQ + s]
            nc.vector.memset(v_sb[:, :, 64 : HPG * 65 : 65], 1.0)

            # prewarm the ACT exp table during the DMA/proj phase
            warm = res.tile([128, 1], F32)
            warm2 = res.tile([128, 1], BF16)
            nc.vector.memset(warm[:], 0.0)
            nc.scalar.activation(warm2[:], warm[:], mybir.ActivationFunctionType.Exp)

            # ================= projections + rope =================
            with tc.tile_pool(name="proj", bufs=2) as proj:
                xts = {}
                for sup in range(NSUP):
                    s0 = sup * 1024
                    xts[sup] = []
                    for kb in range(8):
                        xt = proj.tile([128, 1024], BF16, name=f"xt{sup}_{kb}", tag="xt", bufs=17)
                        nc.sync.dma_start(xt[:], xT_r[:, kb, s0 : s0 + 1024])
                        xts[sup].append(xt)
                        if sup == 0:
                            # interleave the constant streams behind the x tiles,
                            # ordered by first use, so the first projection matmul
                            # starts ~wq+one-tile into the kernel
                            if kb == 0:
                                nc.sync.dma_start(wq_sb[:], wq_d[:])
                            elif kb == 1:
                                nc.sync.dma_start(wk_sb[:], wk_d[:])
                            elif kb == 2:
                                nc.sync.dma_start(rot_sb[:], rot_d[:])
                                nc.sync.dma_start(cos_sb[:], cos_d[:])
                            elif kb == 3:
                                nc.sync.dma_start(sin_sb[:], sin_d[:])
                            elif kb == 4:
                                nc.sync.dma_start(wv_sb[:], wv_d[:])
                            elif kb == 5:
                                nc.sync.dma_start(tri_sb[:], tri_d[:])
                                nc.sync.dma_start(id_sb[:], id_d[:])
                            elif kb == 6:
                                nc.sync.dma_start(wo_sb[:], wo_d[:])

                for sup in range(NSUP):
                    s0 = sup * 1024
                    xp = xts[sup]
                    for tens, (w_sb, outf) in enumerate(((wq_sb, qf), (wk_sb, kf))):
                        # emit both pairs' projection chains before either pair's
                        # rotation, so the rot matmul never blocks the in-order PE
                        # queue waiting on the DVE psum->sbuf copy
                        qraws = []
                        for pr in range(2):
                            qraw = proj.tile([128, 1024], BF16, name="qraw", tag="qraw", bufs=3)
                            for nh in range(2):
                                psq = ps.tile([128, 512], F32, name="psq", tag="pb")
                                for kb in range(8):
                                    lhs = w_sb[:, kb * CPG + pr * 128 : kb * CPG + (pr + 1) * 128]
                                    nc.tensor.matmul(
                                        psq[:],
                                        lhs,
                                        xp[kb][:, nh * 512 : (nh + 1) * 512],
                                        start=(kb == 0),
                                        stop=(kb == 7),
                                    )
                                nc.vector.tensor_copy(qraw[:, nh * 512 : (nh + 1) * 512], psq[:])
                            qraws.append(qraw)
                        for pr in range(2):
                            qraw = qraws[pr]
                            for nh in range(2):
                                psr = ps.tile([128, 512], F32, name="psr", tag="pb")
                                nc.tensor.matmul(
                                    psr[:],
                                    rot_sb[:],
                                    qraw[:, nh * 512 : (nh + 1) * 512],
                                    start=True,
                                    stop=True,
                                )
                                c0 = s0 + nh * 512
                                t1 = proj.tile([128, 512], BF16, name="t1", tag="t1", bufs=3)
                                nc.vector.tensor_mul(
                                    t1[:], qraw[:, nh * 512 : (nh + 1) * 512], cos_sb[:, c0 : c0 + 512]
                                )
                                t2 = proj.tile([128, 512], F32, name="t2", tag="t2", bufs=3)
                                nc.vector.tensor_mul(t2[:], psr[:], sin_sb[:, c0 : c0 + 512])
                                dst = outf[:, pr * SEQ + c0 : pr * SEQ + c0 + 512]
                                nc.vector.tensor_add(dst, t1[:], t2[:])
                    for sbi in range(8):
                        blk = sup * 8 + sbi
                        psv = ps.tile([128, CPG], F32, name="psv", tag="pb")
                        for kb in range(8):
                            nc.tensor.matmul(
                                psv[:],
                                xp[kb][:, sbi * 128 : (sbi + 1) * 128],
                                wv_sb[:, kb * CPG : (kb + 1) * CPG],
                                start=(kb == 0),
                                stop=(kb == 7),
                            )
                        nc.vector.tensor_copy(
                            v_sb[:, blk, :].rearrange("p (h c) -> p h c", h=HPG)[:, :, 0:64],
                            psv[:].rearrange("p (h c) -> p h c", h=HPG),
                        )

            # ================= attention =================
            with tc.tile_pool(name="att", bufs=1) as att:
                EXP = mybir.ActivationFunctionType.Exp

                def emit_scores(h, J):
                    pr, off = h // 2, (h % 2) * 64
                    qT_h = qf[off : off + 64, pr * SEQ : (pr + 1) * SEQ]
                    kT_h = kf[off : off + 64, pr * SEQ : (pr + 1) * SEQ]
                    n_i = 8 * J + 8 if causal else NB
                    ets = []
                    for i in range(n_i):
                        t = i - 8 * J
                        col0 = max(t, 0) * 128 if causal else 0
                        et = att.tile([128, 1024], BF16, name=f"et{h}_{J}_{i}", tag="et", bufs=44)
                        for nh in range(2):
                            lo = max(col0, nh * 512)
                            hi = (nh + 1) * 512
                            if lo >= hi:
                                continue
                            pss = ps.tile([128, 512], F32, name="pss", tag="pb")
                            nc.tensor.matmul(
                                pss[:, 0 : hi - lo],
                                kT_h[:, i * 128 : (i + 1) * 128],
                                qT_h[:, J * 1024 + lo : J * 1024 + hi],
                                start=True,
                                stop=True,
                            )
                            nc.scalar.activation(et[:, lo:hi], pss[:, 0 : hi - lo], EXP)
                        if causal and 0 <= t <= 7:
                            # zero the above-diagonal wedge of the diag block
                            nc.gpsimd.tensor_mul(
                                et[:, t * 128 : (t + 1) * 128],
                                et[:, t * 128 : (t + 1) * 128],
                                tri_sb[:],
                            )
                        ets.append(et)
                    return ets

                def emit_chains(h, J, ets):
                    for jp in range(8):
                        jg = 8 * J + jp
                        n_i = jg + 1 if causal else NB
                        pav = ps.tile([128, 65], F32, name="pav", tag="pav", bufs=3)
                        for i in range(n_i):
                            nc.tensor.matmul(
                                pav[:],
                                ets[i][:, jp * 128 : (jp + 1) * 128],
                                v_sb[:, i, h * 65 : h * 65 + 65],
                                start=(i == 0),
                                stop=(i == n_i - 1),
                            )
                        rec = att.tile([128, 1], F32, name="rec", tag="rec", bufs=4)
                        nc.vector.reciprocal(rec[:], pav[:, 64:65])
                        nc.vector.tensor_scalar_mul(
                            attn[:, jg, h * 64 : (h + 1) * 64], pav[:, 0:64], rec[:]
                        )

                def emit_oproj(J):
                    for jp in range(8):
                        jg = 8 * J + jp
                        for cb in range(2):
                            ptt = ps.tile([128, 128], BF16, name="ptt", tag="pb")
                            nc.tensor.transpose(
                                ptt[:], attn[:, jg, cb * 128 : (cb + 1) * 128], id_sb[:]
                            )
                            nc.vector.tensor_copy(
                                attnT[:, cb * SEQ + jg * 128 : cb * SEQ + (jg + 1) * 128], ptt[:]
                            )
                    for db in range(8):
                        psos = [
                            ps.tile([128, 512], F32, name=f"pso{db}_{ss}", tag="pb")
                            for ss in range(2)
                        ]
                        for cb in range(2):
                            lhs = wo_sb[:, cb * D_MODEL + db * 128 : cb * D_MODEL + (db + 1) * 128]
                            for ss in range(2):
                                nc.tensor.matmul(
                                    psos[ss][:],
                                    lhs,
                                    attnT[
                                        :,
                                        cb * SEQ + J * 1024 + ss * 512 : cb * SEQ
                                        + J * 1024
                                        + (ss + 1) * 512,
                                    ],
                                    start=(cb == 0),
                                    stop=(cb == 1),
                                )
                        osb = att.tile([128, 1024], F32, name="osb", tag="osb", bufs=2)
                        nc.vector.tensor_copy(osb[:, 0:512], psos[0][:])
                        nc.scalar.copy(osb[:, 512:1024], psos[1][:])
                        nc.sync.dma_start(
                            out_d[db * 128 : (db + 1) * 128, J * 1024 : (J + 1) * 1024], osb[:]
                        )

                # J-major schedule: all J0 scores first (they only depend on the
                # first seq-super's rope, so ACT starts exp'ing during sup-1
                # projection); chains of group g interleave with scores of later
                # groups so ACT stays ahead of the PE chain consumers; o_proj of
                # super J overlaps the other super's attention.
                ets0 = [emit_scores(h, 0) for h in range(HPG)]
                ets1 = []
                emit_chains(0, 0, ets0[0])
                ets1.append(emit_scores(0, 1))
                emit_chains(1, 0, ets0[1])
                ets1.append(emit_scores(1, 1))
                emit_chains(2, 0, ets0[2])
                emit_chains(3, 0, ets0[3])
                emit_oproj(0)
                ets1.append(emit_scores(2, 1))
                emit_chains(0, 1, ets1[0])
                ets1.append(emit_scores(3, 1))
                emit_chains(1, 1, ets1[1])
                emit_chains(2, 1, ets1[2])
                emit_chains(3, 1, ets1[3])
                emit_oproj(1)

    nc.compile()
    return nc


def _host_tables():
    inv_freq = 1.0 / (ROPE_THETA ** (np.arange(0, HEAD_DIM, 2, dtype=np.float64) / HEAD_DIM))
    ang = np.arange(SEQ, dtype=np.float64)[:, None] * inv_freq[None, :]  # [S, 32]
    cos_h = np.cos(ang)
    sin_h = np.sin(ang)
    cos_full = np.concatenate([cos_h, cos_h], axis=1).astype(np.float32)  # [S, 64]
    sin_full = np.concatenate([sin_h, sin_h], axis=1).astype(np.float32)
    cosT2 = np.ascontiguousarray(np.vstack([cos_full.T, cos_full.T]))  # [128, S]
    sinT2 = np.ascontiguousarray(np.vstack([sin_full.T, sin_full.T]))
    # rotate_half matrix R [64,64]: (Rq)[j] = -q[j+32] (j<32), q[j-32] (j>=32)
    R = np.zeros((64, 64), np.float32)
    for jj in range(32):
        R[jj, jj + 32] = -1.0
        R[jj + 32, jj] = 1.0
    Rp = np.zeros((128, 128), np.float32)
    Rp[0:64, 0:64] = R
    Rp[64:128, 64:128] = R
    rotT = np.ascontiguousarray(Rp.T)
    return cosT2, sinT2, rotT


def _kb_major(wT):
    # [1024, C] -> [128, 8*C] with K-block-major columns
    C = wT.shape[1]
    return np.ascontiguousarray(wT.reshape(8, 128, C).transpose(1, 0, 2).reshape(128, 8 * C))


def _np_reference(x, mask, Wq, Wk, Wv, Wo):
    B, S, D = x.shape
    cosT2, sinT2, _ = _host_tables()
    cos = cosT2[:64].T[None, :, None, :]  # [1,S,1,64]
    sin = sinT2[:64].T[None, :, None, :]
    q = (x @ Wq.T).reshape(B, S, N_HEADS, HEAD_DIM)
    k = (x @ Wk.T).reshape(B, S, N_HEADS, HEAD_DIM)
    v = (x @ Wv.T).reshape(B, S, N_HEADS, HEAD_DIM)

    def rot(t):
        return np.concatenate([-t[..., 32:], t[..., :32]], axis=-1)

    q = q * cos + rot(q) * sin
    k = k * cos + rot(k) * sin
    sc = np.einsum("bqhd,bkhd->bhqk", q, k) / np.sqrt(HEAD_DIM)
    sc = np.where(mask[None, None], -np.inf, sc)
    sc = sc - sc.max(-1, keepdims=True)
    e = np.exp(sc)
    a = e / e.sum(-1, keepdims=True)
    o = np.einsum("bhqk,bkhd->bqhd", a, v).reshape(B, S, D)
    return (o @ Wo.T).astype(np.float32)


def _bf16(a):
    return np.ascontiguousarray(a).astype(ml_dtypes.bfloat16)


def kernel(x, mask, Wq, Wk, Wv, Wo):
    global LAST_RESULT
    x = np.asarray(x, np.float32)
    mask = np.asarray(mask, bool)
    Wq = np.asarray(Wq, np.float32)
    Wk = np.asarray(Wk, np.float32)
    Wv = np.asarray(Wv, np.float32)
    Wo = np.asarray(Wo, np.float32)

    causal_mask = np.triu(np.ones((SEQ, SEQ), bool), 1)
    if np.array_equal(mask, causal_mask):
        causal = True
    elif not mask.any():
        causal = False
    else:
        return _np_reference(x, mask, Wq, Wk, Wv, Wo)

    if causal not in _CACHE:
        _CACHE[causal] = _build_nc(causal)
    nc = _CACHE[causal]

    cosT2, sinT2, rotT = _host_tables()
    # 0/1 mask for the diagonal 128-block in [k_row, q_col] layout:
    # valid (keep) where q >= k, i.e. col >= row
    tri01 = _bf16(
        np.where(np.arange(128)[None, :] >= np.arange(128)[:, None], 1.0, 0.0).astype(np.float32)
    )
    ident = _bf16(np.eye(128, dtype=np.float32))

    in_maps = []
    for b in range(BATCH):
        xT = _bf16(x[b].T)
        for g in range(4):
            sl = slice(g * CPG, (g + 1) * CPG)
            in_maps.append(
                {
                    "xT": xT,
                    "wqT": _bf16(_kb_major(np.ascontiguousarray((Wq[sl] / np.sqrt(HEAD_DIM)).T))),
                    "wkT": _bf16(_kb_major(np.ascontiguousarray(Wk[sl].T))),
                    "wvT": _bf16(_kb_major(np.ascontiguousarray(Wv[sl].T))),
                    "woT": _bf16(
                        Wo[:, sl].T.reshape(2, 128, D_MODEL).transpose(1, 0, 2).reshape(128, 2 * D_MODEL)
                    ),
                    "cosT2": _bf16(cosT2),
                    "sinT2": _bf16(sinT2),
                    "rotT": _bf16(rotT),
                    "tri01": tri01,
                    "ident": ident,
                }
            )

    trace = os.environ.get("KERNEL_TRACE", "0") == "1"
    res = run_bass_kernel_spmd(nc, in_maps, list(range(8)), trace=trace)
    LAST_RESULT = res

    out = np.zeros((BATCH, SEQ, D_MODEL), np.float32)
    for b in range(BATCH):
        for g in range(4):
            out[b] += res.results[b * 4 + g]["out"].T
    return out


# revision 11
# speedup vs baseline: 1.2158x; 1.0940x over previous
"""Multi-head causal attention with RoPE on 8 Trainium2 NeuronCores.

Sharding: core = batch(2) x head-group(4).  Each core computes the q/k/v
projections for its 4 heads (256 of 1024 channels), RoPE, causal attention,
and a partial o_proj against its 256 rows of Wo^T; the host sums the 4
partials per batch element.

Everything is bf16 on the wire and in SBUF (psum accumulation stays f32):
halves HBM traffic vs f32r and enables PE fast-weight-load.

Device layouts (per core):
  xT       [1024, 2048] bf16   x[b].T
  wqT/wkT/wvT [128, 8*256] bf16  K-block-major W.T slices (wq pre-scaled 1/8)
  woT      [128, 2*1024] bf16  c-block-major Wo[:, g].T
  cosT2/sinT2 [128, 2048] bf16 rope tables, stacked twice (head pair rows)
  rotT     [128, 128]  bf16    blockdiag(R,R).T, R = rotate_half matrix
  tri01/ident [128, 128] bf16  lower-incl-diag 0/1 mask; identity
  out      [2048, 1024] f32    partial (x @ Wo_g partial), host-summed

Attention per head h (Dh=64): scoresT tiles [s_k 128, s_q <=1024] = kT.T@qT,
causal-trimmed at 128 cols, exp -> bf16 et sbuf, diag block masked by a 0/1
triangle multiply on gpsimd.  attn@v runs in NATURAL orientation: per q-block
jg, out[q 128, 65] accumulates matmul(lhsT=et[i][:, q-block], rhs=v[:, i, h])
over k-blocks i<=jg -- exact causal trim, full 128-partition output, ones
column of v carries the softmax denominator.  Normalize straight from psum.
o_proj runs per 1024-seq super, interleaved with the other super's attention.
"""
import os
import sys

sys.path.insert(0, "/opt/trn_rl_repo")

import numpy as np
import ml_dtypes

import concourse.bacc as bacc
import concourse.mybir as mybir
from concourse import tile
from concourse.bass_utils import run_bass_kernel_spmd

F32 = mybir.dt.float32
BF16 = mybir.dt.bfloat16

D_MODEL = 1024
N_HEADS = 16
HEAD_DIM = 64
SEQ = 2048
BATCH = 2
ROPE_THETA = 10000.0

NB = SEQ // 128          # 16 s-blocks of 128
NSUP = SEQ // 1024       # 2 s-supers of 1024
HPG = 4                  # heads per group (per core)
CPG = HPG * HEAD_DIM     # 256 channels per group

_CACHE = {}
LAST_RESULT = None       # test harness reads exec_time_ns from here


def _build_nc(causal: bool):
    nc = bacc.Bacc("TRN2", target_bir_lowering=False, debug=False, num_devices=8)

    xT_d = nc.declare_dram_parameter("xT", [D_MODEL, SEQ], BF16, isOutput=False)
    wq_d = nc.declare_dram_parameter("wqT", [128, 8 * CPG], BF16, isOutput=False)
    wk_d = nc.declare_dram_parameter("wkT", [128, 8 * CPG], BF16, isOutput=False)
    wv_d = nc.declare_dram_parameter("wvT", [128, 8 * CPG], BF16, isOutput=False)
    wo_d = nc.declare_dram_parameter("woT", [128, 2 * D_MODEL], BF16, isOutput=False)
    cos_d = nc.declare_dram_parameter("cosT2", [128, SEQ], BF16, isOutput=False)
    sin_d = nc.declare_dram_parameter("sinT2", [128, SEQ], BF16, isOutput=False)
    rot_d = nc.declare_dram_parameter("rotT", [128, 128], BF16, isOutput=False)
    tri_d = nc.declare_dram_parameter("tri01", [128, 128], BF16, isOutput=False)
    id_d = nc.declare_dram_parameter("ident", [128, 128], BF16, isOutput=False)
    out_d = nc.declare_dram_parameter("out", [D_MODEL, SEQ], BF16, isOutput=True)

    xT_r = xT_d.rearrange("(kb p) s -> p kb s", p=128)

    with tile.TileContext(nc) as tc:
        with (
            tc.tile_pool(name="res", bufs=1) as res,
            tc.tile_pool(name="ps", bufs=5, space="PSUM") as ps,
        ):
            # ---- resident constants ----
            wq_sb = res.tile([128, 8 * CPG], BF16)
            wk_sb = res.tile([128, 8 * CPG], BF16)
            wv_sb = res.tile([128, 8 * CPG], BF16)
            wo_sb = res.tile([128, 2 * D_MODEL], BF16)
            cos_sb = res.tile([128, SEQ], BF16)
            sin_sb = res.tile([128, SEQ], BF16)
            rot_sb = res.tile([128, 128], BF16)
            tri_sb = res.tile([128, 128], BF16)
            id_sb = res.tile([128, 128], BF16)

            # ---- resident activations ----
            qf = res.tile([128, 2 * SEQ], BF16)          # [pair rows, pr*SEQ + s]
            kf = res.tile([128, 2 * SEQ], BF16)
            v_sb = res.tile([128, NB, HPG * 65], BF16)   # per s-block, head-slot 65 cols
            attn = res.tile([128, NB, CPG], BF16)        # attn out, natural [s, c]
            attnT = res.tile([128, 2 * SEQ], BF16)       # attn out transposed [c, cb*SEQ + s]
            nc.vector.memset(v_sb[:, :, 64 : HPG * 65 : 65], 1.0)

            # prewarm the ACT exp table during the DMA/proj phase
            warm = res.tile([128, 1], F32)
            warm2 = res.tile([128, 1], BF16)
            nc.vector.memset(warm[:], 0.0)
            nc.scalar.activation(warm2[:], warm[:], mybir.ActivationFunctionType.Exp)

            # ================= projections + rope =================
            with tc.tile_pool(name="proj", bufs=2) as proj:
                xts = {}
                for sup in range(NSUP):
                    s0 = sup * 1024
                    xts[sup] = []
                    for kb in range(8):
                        xt = proj.tile([128, 1024], BF16, name=f"xt{sup}_{kb}", tag="xt", bufs=17)
                        nc.sync.dma_start(xt[:], xT_r[:, kb, s0 : s0 + 1024])
                        xts[sup].append(xt)
                        if sup == 0:
                            # PE-critical streams first (weights feed the matmul
                            # chains directly); DVE-only tables (cos/sin) and the
                            # late-phase constants trail the x tiles
                            if kb == 0:
                                nc.sync.dma_start(wq_sb[:], wq_d[:])
                            elif kb == 1:
                                nc.sync.dma_start(wk_sb[:], wk_d[:])
                            elif kb == 2:
                                nc.sync.dma_start(rot_sb[:], rot_d[:])
                            elif kb == 3:
                                nc.sync.dma_start(cos_sb[:], cos_d[:])
                            elif kb == 4:
                                nc.sync.dma_start(sin_sb[:], sin_d[:])
                            elif kb == 5:
                                nc.sync.dma_start(wv_sb[:], wv_d[:])
                            elif kb == 6:
                                nc.sync.dma_start(tri_sb[:], tri_d[:])
                                nc.sync.dma_start(id_sb[:], id_d[:])
                            elif kb == 7:
                                nc.sync.dma_start(wo_sb[:], wo_d[:])

                # q/k chains for both supers first; v projections afterwards so
                # the PE never waits on the later-arriving wv stream
                for sup in range(NSUP):
                    s0 = sup * 1024
                    xp = xts[sup]
                    for tens, (w_sb, outf) in enumerate(((wq_sb, qf), (wk_sb, kf))):
                        # emit both pairs' projection chains before either pair's
                        # rotation, so the rot matmul never blocks the in-order PE
                        # queue waiting on the DVE psum->sbuf copy
                        qraws = []
                        for pr in range(2):
                            qraw = proj.tile([128, 1024], BF16, name="qraw", tag="qraw", bufs=3)
                            for nh in range(2):
                                psq = ps.tile([128, 512], F32, name="psq", tag="pb")
                                for kb in range(8):
                                    lhs = w_sb[:, kb * CPG + pr * 128 : kb * CPG + (pr + 1) * 128]
                                    nc.tensor.matmul(
                                        psq[:],
                                        lhs,
                                        xp[kb][:, nh * 512 : (nh + 1) * 512],
                                        start=(kb == 0),
                                        stop=(kb == 7),
                                    )
                                nc.vector.tensor_copy(qraw[:, nh * 512 : (nh + 1) * 512], psq[:])
                            qraws.append(qraw)
                        for pr in range(2):
                            qraw = qraws[pr]
                            for nh in range(2):
                                psr = ps.tile([128, 512], F32, name="psr", tag="pb")
                                nc.tensor.matmul(
                                    psr[:],
                                    rot_sb[:],
                                    qraw[:, nh * 512 : (nh + 1) * 512],
                                    start=True,
                                    stop=True,
                                )
                                c0 = s0 + nh * 512
                                t1 = proj.tile([128, 512], BF16, name="t1", tag="t1", bufs=3)
                                nc.vector.tensor_mul(
                                    t1[:], qraw[:, nh * 512 : (nh + 1) * 512], cos_sb[:, c0 : c0 + 512]
                                )
                                t2 = proj.tile([128, 512], F32, name="t2", tag="t2", bufs=3)
                                nc.vector.tensor_mul(t2[:], psr[:], sin_sb[:, c0 : c0 + 512])
                                dst = outf[:, pr * SEQ + c0 : pr * SEQ + c0 + 512]
                                nc.vector.tensor_add(dst, t1[:], t2[:])
                for sup in range(NSUP):
                    xp = xts[sup]
                    for sbi in range(8):
                        blk = sup * 8 + sbi
                        psv = ps.tile([128, CPG], F32, name="psv", tag="pb")
                        for kb in range(8):
                            nc.tensor.matmul(
                                psv[:],
                                xp[kb][:, sbi * 128 : (sbi + 1) * 128],
                                wv_sb[:, kb * CPG : (kb + 1) * CPG],
                                start=(kb == 0),
                                stop=(kb == 7),
                            )
                        nc.vector.tensor_copy(
                            v_sb[:, blk, :].rearrange("p (h c) -> p h c", h=HPG)[:, :, 0:64],
                            psv[:].rearrange("p (h c) -> p h c", h=HPG),
                        )

            # ================= attention =================
            with tc.tile_pool(name="att", bufs=1) as att:
                EXP = mybir.ActivationFunctionType.Exp

                def emit_scores_pair(p, J):
                    # Heads 2p (partitions 0:64) and 2p+1 (partitions 64:128)
                    # run as concurrent 64x128 row tiles of the PE array:
                    # tile_position (0,0) computes on array rows 0-63 while
                    # (64,0) computes on rows 64-127 -- 2x score throughput.
                    n_i = 8 * J + 8 if causal else NB
                    ets = ([], [])
                    for i in range(n_i):
                        t = i - 8 * J
                        col0 = max(t, 0) * 128 if causal else 0
                        pair = []
                        for hh in range(2):
                            h = 2 * p + hh
                            et = att.tile(
                                [128, 1024], BF16, name=f"et{h}_{J}_{i}", tag="et", bufs=64
                            )
                            pair.append(et)
                            ets[hh].append(et)
                        psss = ([], [])
                        for nh in range(2):
                            lo = max(col0, nh * 512)
                            hi = (nh + 1) * 512
                            if lo >= hi:
                                continue
                            for hh in range(2):
                                off = hh * 64
                                qT_h = qf[off : off + 64, p * SEQ : (p + 1) * SEQ]
                                kT_h = kf[off : off + 64, p * SEQ : (p + 1) * SEQ]
                                pss = ps.tile([128, 512], F32, name="pss", tag="pb")
                                nc.tensor.matmul(
                                    pss[:, 0 : hi - lo],
                                    kT_h[:, i * 128 : (i + 1) * 128],
                                    qT_h[:, J * 1024 + lo : J * 1024 + hi],
                                    start=True,
                                    stop=True,
                                    tile_position=(off, 0),
                                )
                                psss[hh].append((pss, lo, hi))
                        for hh in range(2):
                            for pss, lo, hi in psss[hh]:
                                nc.scalar.activation(
                                    pair[hh][:, lo:hi], pss[:, 0 : hi - lo], EXP
                                )
                            if causal and 0 <= t <= 7:
                                # zero the above-diagonal wedge of the diag block
                                nc.gpsimd.tensor_mul(
                                    pair[hh][:, t * 128 : (t + 1) * 128],
                                    pair[hh][:, t * 128 : (t + 1) * 128],
                                    tri_sb[:],
                                )
                    return ets

                def emit_chains(h, J, ets):
                    for jp in range(8):
                        jg = 8 * J + jp
                        n_i = jg + 1 if causal else NB
                        pav = ps.tile([128, 65], F32, name="pav", tag="pav", bufs=3)
                        for i in range(n_i):
                            nc.tensor.matmul(
                                pav[:],
                                ets[i][:, jp * 128 : (jp + 1) * 128],
                                v_sb[:, i, h * 65 : h * 65 + 65],
                                start=(i == 0),
                                stop=(i == n_i - 1),
                            )
                        rec = att.tile([128, 1], F32, name="rec", tag="rec", bufs=4)
                        nc.vector.reciprocal(rec[:], pav[:, 64:65])
                        nc.vector.tensor_scalar_mul(
                            attn[:, jg, h * 64 : (h + 1) * 64], pav[:, 0:64], rec[:]
                        )

                def emit_oproj(J):
                    for jp in range(8):
                        jg = 8 * J + jp
                        for cb in range(2):
                            ptt = ps.tile([128, 128], BF16, name="ptt", tag="pb")
                            nc.tensor.transpose(
                                ptt[:], attn[:, jg, cb * 128 : (cb + 1) * 128], id_sb[:]
                            )
                            nc.vector.tensor_copy(
                                attnT[:, cb * SEQ + jg * 128 : cb * SEQ + (jg + 1) * 128], ptt[:]
                            )
                    for db in range(8):
                        psos = [
                            ps.tile([128, 512], F32, name=f"pso{db}_{ss}", tag="pb")
                            for ss in range(2)
                        ]
                        for cb in range(2):
                            lhs = wo_sb[:, cb * D_MODEL + db * 128 : cb * D_MODEL + (db + 1) * 128]
                            for ss in range(2):
                                nc.tensor.matmul(
                                    psos[ss][:],
                                    lhs,
                                    attnT[
                                        :,
                                        cb * SEQ + J * 1024 + ss * 512 : cb * SEQ
                                        + J * 1024
                                        + (ss + 1) * 512,
                                    ],
                                    start=(cb == 0),
                                    stop=(cb == 1),
                                )
                        osb = att.tile([128, 1024], BF16, name="osb", tag="osb", bufs=2)
                        nc.vector.tensor_copy(osb[:, 0:512], psos[0][:])
                        nc.scalar.copy(osb[:, 512:1024], psos[1][:])
                        nc.sync.dma_start(
                            out_d[db * 128 : (db + 1) * 128, J * 1024 : (J + 1) * 1024], osb[:]
                        )

                # J-major schedule: score pair-batches lead their chain
                # consumers by enough for ACT exp to stay ahead of the PE;
                # o_proj of super 0 overlaps super-1 attention.  With et
                # bufs=64 the slot-reuse chain never crosses a consumer that
                # is emitted later (verified: slot k+64's consumer always
                # precedes allocation k+64 in PE order).
                e00, e10 = emit_scores_pair(0, 0)   # heads 0,1  J0
                e20, e30 = emit_scores_pair(1, 0)   # heads 2,3  J0
                emit_chains(0, 0, e00)
                e01, e11 = emit_scores_pair(0, 1)   # heads 0,1  J1
                emit_chains(1, 0, e10)
                emit_chains(2, 0, e20)
                emit_chains(3, 0, e30)
                e21, e31 = emit_scores_pair(1, 1)   # heads 2,3  J1
                emit_oproj(0)
                emit_chains(0, 1, e01)
                emit_chains(1, 1, e11)
                emit_chains(2, 1, e21)
                emit_chains(3, 1, e31)
                emit_oproj(1)

    nc.compile()
    return nc


def _host_tables():
    inv_freq = 1.0 / (ROPE_THETA ** (np.arange(0, HEAD_DIM, 2, dtype=np.float64) / HEAD_DIM))
    ang = np.arange(SEQ, dtype=np.float64)[:, None] * inv_freq[None, :]  # [S, 32]
    cos_h = np.cos(ang)
    sin_h = np.sin(ang)
    cos_full = np.concatenate([cos_h, cos_h], axis=1).astype(np.float32)  # [S, 64]
    sin_full = np.concatenate([sin_h, sin_h], axis=1).astype(np.float32)
    cosT2 = np.ascontiguousarray(np.vstack([cos_full.T, cos_full.T]))  # [128, S]
    sinT2 = np.ascontiguousarray(np.vstack([sin_full.T, sin_full.T]))
    # rotate_half matrix R [64,64]: (Rq)[j] = -q[j+32] (j<32), q[j-32] (j>=32)
    R = np.zeros((64, 64), np.float32)
    for jj in range(32):
        R[jj, jj + 32] = -1.0
        R[jj + 32, jj] = 1.0
    Rp = np.zeros((128, 128), np.float32)
    Rp[0:64, 0:64] = R
    Rp[64:128, 64:128] = R
    rotT = np.ascontiguousarray(Rp.T)
    return cosT2, sinT2, rotT


def _kb_major(wT):
    # [1024, C] -> [128, 8*C] with K-block-major columns
    C = wT.shape[1]
    return np.ascontiguousarray(wT.reshape(8, 128, C).transpose(1, 0, 2).reshape(128, 8 * C))


def _np_reference(x, mask, Wq, Wk, Wv, Wo):
    B, S, D = x.shape
    cosT2, sinT2, _ = _host_tables()
    cos = cosT2[:64].T[None, :, None, :]  # [1,S,1,64]
    sin = sinT2[:64].T[None, :, None, :]
    q = (x @ Wq.T).reshape(B, S, N_HEADS, HEAD_DIM)
    k = (x @ Wk.T).reshape(B, S, N_HEADS, HEAD_DIM)
    v = (x @ Wv.T).reshape(B, S, N_HEADS, HEAD_DIM)

    def rot(t):
        return np.concatenate([-t[..., 32:], t[..., :32]], axis=-1)

    q = q * cos + rot(q) * sin
    k = k * cos + rot(k) * sin
    sc = np.einsum("bqhd,bkhd->bhqk", q, k) / np.sqrt(HEAD_DIM)
    sc = np.where(mask[None, None], -np.inf, sc)
    sc = sc - sc.max(-1, keepdims=True)
    e = np.exp(sc)
    a = e / e.sum(-1, keepdims=True)
    o = np.einsum("bhqk,bkhd->bqhd", a, v).reshape(B, S, D)
    return (o @ Wo.T).astype(np.float32)


def _bf16(a):
    return np.ascontiguousarray(a).astype(ml_dtypes.bfloat16)


def kernel(x, mask, Wq, Wk, Wv, Wo):
    global LAST_RESULT
    x = np.asarray(x, np.float32)
    mask = np.asarray(mask, bool)
    Wq = np.asarray(Wq, np.float32)
    Wk = np.asarray(Wk, np.float32)
    Wv = np.asarray(Wv, np.float32)
    Wo = np.asarray(Wo, np.float32)

    causal_mask = np.triu(np.ones((SEQ, SEQ), bool), 1)
    if np.array_equal(mask, causal_mask):
        causal = True
    elif not mask.any():
        causal = False
    else:
        return _np_reference(x, mask, Wq, Wk, Wv, Wo)

    if causal not in _CACHE:
        _CACHE[causal] = _build_nc(causal)
    nc = _CACHE[causal]

    cosT2, sinT2, rotT = _host_tables()
    # 0/1 mask for the diagonal 128-block in [k_row, q_col] layout:
    # valid (keep) where q >= k, i.e. col >= row
    tri01 = _bf16(
        np.where(np.arange(128)[None, :] >= np.arange(128)[:, None], 1.0, 0.0).astype(np.float32)
    )
    ident = _bf16(np.eye(128, dtype=np.float32))

    in_maps = []
    for b in range(BATCH):
        xT = _bf16(x[b].T)
        for g in range(4):
            sl = slice(g * CPG, (g + 1) * CPG)
            in_maps.append(
                {
                    "xT": xT,
                    "wqT": _bf16(_kb_major(np.ascontiguousarray((Wq[sl] / np.sqrt(HEAD_DIM)).T))),
                    "wkT": _bf16(_kb_major(np.ascontiguousarray(Wk[sl].T))),
                    "wvT": _bf16(_kb_major(np.ascontiguousarray(Wv[sl].T))),
                    "woT": _bf16(
                        Wo[:, sl].T.reshape(2, 128, D_MODEL).transpose(1, 0, 2).reshape(128, 2 * D_MODEL)
                    ),
                    "cosT2": _bf16(cosT2),
                    "sinT2": _bf16(sinT2),
                    "rotT": _bf16(rotT),
                    "tri01": tri01,
                    "ident": ident,
                }
            )

    trace = os.environ.get("KERNEL_TRACE", "0") == "1"
    res = run_bass_kernel_spmd(nc, in_maps, list(range(8)), trace=trace)
    LAST_RESULT = res

    out = np.zeros((BATCH, SEQ, D_MODEL), np.float32)
    for b in range(BATCH):
        for g in range(4):
            out[b] += res.results[b * 4 + g]["out"].astype(np.float32).T
    return out
